# revision 13
# baseline (speedup 1.0000x reference)
"""DANetHead Trainium2 kernel: 8-core SPMD (batch x row-half sharding).

Self-contained: hardcodes all shapes from the problem spec.

Per-core layout (core c: sample b=c//2, half h=c%2):
  P = [-1, 0..63, 64] (66 padded rows; -1/64 zero).
  x_pad rows R=0..67 hold padded row P[(R-1+32h) % 66]  (cyclic rotation, so
  every core's attention/conv2 window is local rows 0..33 uniformly).
  conv1 output local row L (0..65) centers on P[(L+32h) % 66].
  window = local rows 0..33 (flat 0..2175); my output rows = 1..32.
"""
import numpy as np

import concourse.bass as bass
import concourse.tile as tile
from concourse import bacc, mybir
from concourse.bass_utils import run_bass_kernel_spmd

F32 = mybir.dt.float32
F32R = mybir.dt.float32r
AF = mybir.ActivationFunctionType
ALU = mybir.AluOpType

B, CIN, H, W = 4, 256, 64, 64
CI, CQ, CO = 64, 8, 256
NCORES = 8
LR = 66                  # local feat1 rows
NP = LR * W              # 4224
NJT = NP // 128          # 33 j-tiles
WIN = 34 * W             # 2176
MY = 32 * W              # 2048
XR, XC = 68, 66          # x_pad rows/cols
NTAPS = 18               # 9 taps x 2 cin blocks
# i chunks for attention window
IC = [(0, 512), (512, 512), (1024, 512), (1536, 512), (2048, 128)]
# conv1 output tiles: (row0, nrows, chunk)
C1T = [(8 * T, 8, T) for T in range(8)] + [(64, 2, 8)]
C1GRP = [(0, 1), (2, 3), (4, 5), (6, 7, 8)]
XCHUNK = [(8 * T, 10) for T in range(8)] + [(64, 4)]  # (row0, nrows)
N_STAT = 16384.0


# ---------------------------------------------------------------- host prep
def _rot_centers(h):
    P = [-1] + list(range(64)) + [64]
    return [P[(L + 32 * h) % 66] for L in range(LR)]


def _prep_core_inputs(x, w1, bn_g, bn_b, wq, bq, wk, bk, wv, bv,
                      gamma_pam, gamma_cam, w2, w8, b8):
    f = np.float32
    # shared weights
    w1s = np.zeros((128, NTAPS, CI), f)
    for dy in range(3):
        for dx in range(3):
            for cb in range(2):
                s = (dy * 3 + dx) * 2 + cb
                w1s[:, s, :] = w1[:, cb * 128:(cb + 1) * 128, dy, dx].T
    wqkv = np.zeros((65, 80), f)
    wqkv[:64, 0:64] = wv[:, :, 0, 0].T
    wqkv[:64, 64:72] = wq[:, :, 0, 0].T
    wqkv[:64, 72:80] = wk[:, :, 0, 0].T
    wqkv[64, 0:64] = bv
    wqkv[64, 64:72] = bq
    wqkv[64, 72:80] = bk
    w2a = np.zeros((128, 3, CI), f)
    w2b = np.zeros((64, 3, CI), f)
    for dx in range(3):
        w2a[:64, dx, :] = w2[:, :, 0, dx].T
        w2a[64:, dx, :] = w2[:, :, 1, dx].T
        w2b[:, dx, :] = w2[:, :, 2, dx].T
    w8s = np.zeros((65, 2, 128), f)
    for blk in range(2):
        w8s[:64, blk, :] = w8[blk * 128:(blk + 1) * 128, :, 0, 0].T
        w8s[64, blk, :] = b8[blk * 128:(blk + 1) * 128]
    bngb = np.stack([bn_g, bn_b], 1).astype(f)
    consts = np.array([[float(gamma_pam[0]), float(gamma_cam[0])]], f)
    iden = np.eye(128, dtype=f)

    shared = dict(w1s=w1s, wqkv=wqkv, w2a=w2a.reshape(128, 3 * CI),
                  w2b=w2b.reshape(64, 3 * CI), w8s=w8s.reshape(65, 256),
                  bngb=bngb, consts=consts, iden=iden)

    in_maps = []
    for c in range(NCORES):
        b, h = divmod(c, 2)
        # x_pad [128, 2, 68, 66]
        P = [-1] + list(range(64)) + [64]
        rows = [P[(R - 1 + 32 * h) % 66] for R in range(XR)]
        xp = np.zeros((128, 2, XR, XC), f)
        for R, g in enumerate(rows):
            if 0 <= g <= 63:
                xr = x[b, :, g, :]                       # [256, 64]
                xp[:, 0, R, 1:65] = xr[:128]
                xp[:, 1, R, 1:65] = xr[128:]
        centers = _rot_centers(h)
        real = np.array([0 <= g <= 63 for g in centers])
        realp = np.repeat(real, W)                        # [4224]
        ebias = np.where(realp, 0.0, -1000.0).astype(f).reshape(NJT, 128).T.copy()
        nmask = np.where(realp, 1.0, 0.0).astype(f).reshape(NJT, 128).T.copy()
        hmask = np.zeros((64, 2), f)
        hmask[:, 0] = 0.0 if h == 0 else 1.0
        hmask[:, 1] = 0.0 if h == 1 else 1.0
        m = dict(shared)
        m.update(xp=xp, ebias=ebias, nmask=nmask, hmask=hmask)
        in_maps.append(m)
    return in_maps


# ---------------------------------------------------------------- bass build
def _build():
    nc = bacc.Bacc()
    xp = nc.declare_dram_parameter("xp", [128, 2, XR, XC], F32R, isOutput=False)
    w1s = nc.declare_dram_parameter("w1s", [128, NTAPS, CI], F32R, isOutput=False)
    wqkv = nc.declare_dram_parameter("wqkv", [65, 80], F32R, isOutput=False)
    w2a = nc.declare_dram_parameter("w2a", [128, 3 * CI], F32R, isOutput=False)
    w2b = nc.declare_dram_parameter("w2b", [64, 3 * CI], F32R, isOutput=False)
    w8s = nc.declare_dram_parameter("w8s", [65, 256], F32R, isOutput=False)
    bngb = nc.declare_dram_parameter("bngb", [64, 2], F32, isOutput=False)
    ebias = nc.declare_dram_parameter("ebias", [128, NJT], F32, isOutput=False)
    nmask = nc.declare_dram_parameter("nmask", [128, NJT], F32, isOutput=False)
    hmask = nc.declare_dram_parameter("hmask", [64, 2], F32, isOutput=False)
    consts = nc.declare_dram_parameter("consts", [1, 2], F32, isOutput=False)
    iden = nc.declare_dram_parameter("iden", [128, 128], F32R, isOutput=False)
    out = nc.declare_dram_parameter("out", [256, MY], F32, isOutput=True)

    with tile.TileContext(nc) as tc:
        with tc.tile_pool(name="big", bufs=1) as big, \
             tc.tile_pool(name="xin", bufs=2) as xin, \
             tc.tile_pool(name="wt", bufs=1) as wt, \
             tc.tile_pool(name="sm", bufs=8) as sm, \
             tc.tile_pool(name="et", bufs=4) as etp, \
             tc.tile_pool(name="ob", bufs=1) as obp, \
             tc.tile_pool(name="ps", bufs=3, space="PSUM") as ps, \
             tc.tile_pool(name="pam", bufs=5, space="PSUM") as pp, \
             tc.tile_pool(name="dram", bufs=1, space="DRAM") as dram:

            # ---- persistent sbuf tensors
            feat = big.tile([65, NP], F32R, tag="feat")      # y1 then feat1(+ones)
            qkv = big.tile([80, NP], F32R, tag="qkv")
            qr = big.tile([128, WIN], F32R, tag="qr")
            kr4 = big.tile([128, 9, 128], F32R, tag="kr4")
            vT = big.tile([128, NJT, 65], F32R, tag="vT")
            fT = big.tile([128, NJT, CI], F32R, tag="fT")
            sabuf = big.tile([128, 34, XC], F32R, tag="sabuf")
            scbuf = big.tile([128, 34, XC], F32R, tag="scbuf")
            y2a = big.tile([64, MY], F32, tag="y2a")
            y2b = big.tile([64, MY], F32, tag="y2b")
            fsum = big.tile([65, MY], F32R, tag="fsum")

            # ---- weights / consts in sbuf
            w1t = wt.tile([128, NTAPS, CI], F32R, tag="w1t")
            wqkvt = wt.tile([65, 80], F32R, tag="wqkvt")
            w2at = wt.tile([128, 3 * CI], F32R, tag="w2at")
            w2bt = wt.tile([64, 3 * CI], F32R, tag="w2bt")
            w8t = wt.tile([65, 256], F32R, tag="w8t")
            bngbt = wt.tile([64, 2], F32, tag="bngbt")
            ebt = wt.tile([128, NJT], F32, tag="ebt")
            nmt = wt.tile([128, NJT], F32, tag="nmt")
            hmt = wt.tile([64, 2], F32, tag="hmt")
            cst = wt.tile([1, 2], F32, tag="cst")
            gcam = wt.tile([64, 1], F32, tag="gcam")
            epst = wt.tile([64, 1], F32, tag="epst")
            nc.vector.memset(epst, 1e-5)
            idt = wt.tile([128, 128], F32R, tag="idt")
            nc.sync.dma_start(out=w1t, in_=w1s[:, :, :])
            nc.sync.dma_start(out=wqkvt, in_=wqkv[:, :])
            nc.sync.dma_start(out=w2at, in_=w2a[:, :])
            nc.sync.dma_start(out=w2bt, in_=w2b[:, :])
            nc.sync.dma_start(out=w8t, in_=w8s[:, :])
            nc.sync.dma_start(out=bngbt, in_=bngb[:, :])
            nc.sync.dma_start(out=ebt, in_=ebias[:, :])
            nc.sync.dma_start(out=nmt, in_=nmask[:, :])
            nc.sync.dma_start(out=hmt, in_=hmask[:, :])
            nc.sync.dma_start(out=cst, in_=consts[:, :])
            nc.sync.dma_start(out=idt, in_=iden[:, :])
            # broadcast gamma_cam to [64,1]
            gc_src = bass.AP(tensor=consts, offset=1,
                             ap=[[0, 64], [1, 1]])
            nc.gpsimd.dma_start(out=gcam, in_=gc_src)
            nc.vector.memset(feat[64:65, :].bitcast(F32), 1.0)
            nc.vector.memset(fsum[64:65, :].bitcast(F32), 1.0)
            nc.vector.memset(qr[:, :].bitcast(F32), 0.0)
            nc.vector.memset(kr4[:, :, :].bitcast(F32), 0.0)
            nc.vector.memset(vT[:, :, 64:65].bitcast(F32), 1.0)
            nc.gpsimd.memset(sabuf[:, :, :].bitcast(F32), 0.0)
            nc.gpsimd.memset(scbuf[:, :, :].bitcast(F32), 0.0)

            # ---- x chunks
            xc = []
            for (r0, nr) in XCHUNK:
                t = xin.tile([128, 2, nr, XC], F32R, tag=f"xc{nr}",
                             name=f"xc{r0}", bufs=3 if nr == 10 else 1)
                nc.sync.dma_start(out=t, in_=xp[:, :, r0:r0 + nr, :])
                xc.append(t)

            # ---- conv1 -> feat rows 0..63 hold raw y1
            stats1 = sm.tile([64, 5, 6], F32, tag="stats1")
            stat_slices = [(0, 64, 448), (1, 0, 512), (2, 0, 512),
                           (3, 0, 512), (4, 0, 64)]
            for grp in C1GRP:
                pst = {}
                for T in grp:
                    r0, nr, ci_ = C1T[T]
                    pst[T] = ps.tile([64, nr * W], F32, tag="ps", name=f"c1ps{T}")
                for s in range(NTAPS):
                    tap, cb = divmod(s, 2)
                    dy, dx = divmod(tap, 3)
                    for T in grp:
                        r0, nr, ci_ = C1T[T]
                        rhs = xc[ci_][:, cb, dy:dy + nr, dx:dx + 64]
                        nc.tensor.matmul(pst[T], w1t[:, s, :], rhs,
                                         start=(s == 0), stop=(s == NTAPS - 1))
                for T in grp:
                    r0, nr, ci_ = C1T[T]
                    nc.vector.tensor_copy(feat[0:64, r0 * W:(r0 + nr) * W], pst[T])
            for (k, off, ln) in stat_slices:
                T0 = [0, 512, 1024, 1536, 2048][k]
                nc.vector.bn_stats(stats1[:, k, :], feat[0:64, T0 + off:T0 + off + ln])
            mv1 = sm.tile([64, 2], F32, tag="mv1")
            nc.vector.bn_aggr(mv1, stats1[:, :, :])

            # partial sums -> AR1
            ar1s = sm.tile([64, 2], F32, tag="ar1s")
            t_a = sm.tile([64, 1], F32, tag="t_a")
            nc.vector.tensor_scalar_mul(ar1s[:, 0:1], mv1[:, 0:1], float(MY))
            nc.vector.tensor_tensor(t_a, mv1[:, 0:1], mv1[:, 0:1], ALU.mult)
            nc.vector.tensor_tensor(t_a, mv1[:, 1:2], t_a, ALU.add)
            nc.vector.tensor_scalar_mul(ar1s[:, 1:2], t_a, float(MY))
            ar1_in = dram.tile([64, 2], F32, tag="ar1_in")
            ar1_out = dram.tile([64, 2], F32, tag="ar1_out")
            nc.sync.dma_start(out=ar1_in[:, :], in_=ar1s)
            nc.gpsimd.collective_compute(
                "AllReduce", ALU.add, replica_groups=[list(range(NCORES))],
                ins=[ar1_in.opt()], outs=[ar1_out.opt()])
            gl1 = sm.tile([64, 2], F32, tag="gl1")
            nc.sync.dma_start(out=gl1, in_=ar1_out[:, :])

            def bn_coeffs(gl, tag):
                """gl [64,2] = (sum, sumsq) -> (scale, shift) [64,1] f32."""
                mean = sm.tile([64, 1], F32, tag=tag + "m")
                var = sm.tile([64, 1], F32, tag=tag + "v")
                scl = sm.tile([64, 1], F32, tag=tag + "s")
                sh = sm.tile([64, 1], F32, tag=tag + "h")
                nc.vector.tensor_scalar_mul(mean, gl[:, 0:1], 1.0 / N_STAT)
                nc.vector.tensor_scalar_mul(var, gl[:, 1:2], 1.0 / N_STAT)
                nc.vector.tensor_tensor(scl, mean, mean, ALU.mult)
                nc.vector.tensor_tensor(var, var, scl, ALU.subtract)
                nc.scalar.activation(var, var, AF.Sqrt, bias=epst, scale=1.0)
                nc.vector.reciprocal(var, var)
                nc.vector.tensor_tensor(scl, bngbt[:, 0:1], var, ALU.mult)
                nc.vector.tensor_tensor(sh, mean, scl, ALU.mult)
                nc.vector.tensor_tensor(sh, bngbt[:, 1:2], sh, ALU.subtract)
                return scl, sh

            sc1, sh1 = bn_coeffs(gl1, "bn1")
            # feat1 = relu(y1*sc1+sh1) in place
            for (r0, nr, _) in C1T:
                sl = feat[0:64, r0 * W:(r0 + nr) * W]
                nc.scalar.activation(sl, sl, AF.Relu, bias=sh1, scale=sc1)

            # ---- qkv
            qkvtiles = [(0, 512), (512, 512), (1024, 512), (1536, 512),
                        (2048, 512), (2560, 512), (3072, 512), (3584, 512),
                        (4096, 128)]
            for ti, (c0, cw) in enumerate(qkvtiles):
                qps = ps.tile([80, cw], F32, tag="ps")
                nc.tensor.matmul(qps, wqkvt, feat[:, c0:c0 + cw],
                                 start=True, stop=True)
                nc.vector.tensor_copy(qkv[:, c0:c0 + cw], qps)
            # qr: q replicated at partition groups (window cols only)
            for g in range(4):
                for (c0, cw) in [(0, 512), (512, 512), (1024, 512),
                                 (1536, 512), (2048, 128)]:
                    nc.sync.dma_start(out=qr[32 * g:32 * g + 8, c0:c0 + cw],
                                      in_=qkv[64:72, c0:c0 + cw])
            # kr4: k repartitioned per j-group
            for t in range(9):
                nu = 4 if t < 8 else 1
                for u in range(nu):
                    j0 = t * 512 + u * 128
                    nc.sync.dma_start(out=kr4[32 * u:32 * u + 8, t, :],
                                      in_=qkv[72:80, j0:j0 + 128])

            # ---- transposes: vT (with ones col), fT (masked)
            for jt in range(NJT):
                tp = ps.tile([128, 64], F32R, tag="ps", name=f"vtp{jt}")
                nc.tensor.transpose(tp, qkv[0:64, jt * 128:(jt + 1) * 128],
                                    idt[0:64, 0:64])
                nc.vector.tensor_copy(vT[:, jt, 0:64], tp)
            for jt in range(NJT):
                tp = ps.tile([128, 64], F32R, tag="ps", name=f"ftp{jt}")
                nc.tensor.transpose(tp, feat[0:64, jt * 128:(jt + 1) * 128],
                                    idt[0:64, 0:64])
                nc.vector.tensor_scalar_mul(fT[:, jt, :], tp, nmt[:, jt:jt + 1])

            # ---- CAM (emitted early; overlaps PAM on free engines)
            ce_ps = ps.tile([64, 64], F32, tag="ps")
            for jt in range(NJT):
                nc.tensor.matmul(ce_ps, fT[:, jt, :], fT[:, jt, :],
                                 start=(jt == 0), stop=(jt == NJT - 1))
            rmin = sm.tile([64, 1], F32, tag="rmin")
            nc.vector.tensor_reduce(rmin, ce_ps, mybir.AxisListType.X, ALU.min)
            cu = sm.tile([64, 64], F32, tag="cu")
            nc.scalar.activation(cu, ce_ps, AF.Exp, bias=rmin, scale=-1.0)
            rs = sm.tile([64, 1], F32, tag="rs")
            nc.vector.tensor_reduce(rs, cu, mybir.AxisListType.X, ALU.add)
            nc.vector.reciprocal(rs, rs)
            cattn = sm.tile([64, 64], F32R, tag="cattn")
            nc.vector.tensor_scalar_mul(cattn, cu, rs)
            ctp = ps.tile([64, 64], F32R, tag="ps")
            nc.tensor.transpose(ctp, cattn, idt[0:64, 0:64])
            cattnT = sm.tile([64, 64], F32R, tag="cattnT")
            nc.vector.tensor_copy(cattnT, ctp)
            for (i0, iw) in IC:
                cam_ps = ps.tile([64, iw], F32, tag="ps")
                nc.tensor.matmul(cam_ps, cattnT, feat[0:64, i0:i0 + iw],
                                 start=True, stop=True)
                tmpc = etp.tile([64, iw], F32R, tag="camt", bufs=2)
                nc.vector.tensor_scalar_mul(tmpc, cam_ps, gcam)
                r0, nr = i0 // W, iw // W
                nc.vector.tensor_tensor(
                    scbuf[0:64, r0:r0 + nr, 1:65],
                    tmpc[:, :].rearrange("p (r c) -> p r c", c=W),
                    feat[0:64, i0:i0 + iw].rearrange("p (r c) -> p r c", c=W),
                    ALU.add)
            nc.vector.tensor_scalar_mul(scbuf[0:64, 0, 1:65],
                                        scbuf[0:64, 0, 1:65], hmt[:, 0:1])
            nc.vector.tensor_scalar_mul(scbuf[0:64, 33, 1:65],
                                        scbuf[0:64, 33, 1:65], hmt[:, 1:2])
            nc.sync.dma_start(out=scbuf[64:128, 0:33, :],
                              in_=scbuf[0:64, 1:34, :])

            def conv2(buf, y2sb, sttag):
                st = sm.tile([64, 4, 6], F32, tag=sttag)
                for T in range(4):
                    r0 = 1 + 8 * T
                    yps = ps.tile([64, 512], F32, tag="ps")
                    for dxi in range(3):
                        first = dxi == 0
                        last = dxi == 2
                        rhs1 = buf[:, r0 - 1:r0 + 7, dxi:dxi + 64]
                        nc.tensor.matmul(yps, w2at[:, dxi * 64:(dxi + 1) * 64],
                                         rhs1, start=first, stop=False)
                        rhs2 = buf[0:64, r0 + 1:r0 + 9, dxi:dxi + 64]
                        nc.tensor.matmul(yps, w2bt[:, dxi * 64:(dxi + 1) * 64],
                                         rhs2, start=False, stop=last)
                    nc.vector.bn_stats(st[:, T, :], yps)
                    nc.vector.tensor_copy(y2sb[:, T * 512:(T + 1) * 512], yps)
                mv = sm.tile([64, 2], F32, tag=sttag + "mv")
                nc.vector.bn_aggr(mv, st[:, :, :])
                return mv

            mvb = conv2(scbuf, y2b, "stb")

            # ---- PAM attention
            pamt = [pp.tile([65, iw], F32, tag="pam", name=f"pam{i}")
                    for i, (_, iw) in enumerate(IC)]
            for jg in range(9):
                nu = 4 if jg < 8 else 1
                for ici, (i0, iw) in enumerate(IC):
                    for u in range(nu):
                        jt = 4 * jg + u
                        eps_ = ps.tile([128, iw], F32, tag="ps")
                        nc.tensor.matmul(eps_, kr4[32 * u:32 * u + 32, jg, :],
                                         qr[32 * u:32 * u + 32, i0:i0 + iw],
                                         start=True, stop=True,
                                         tile_position=(32 * u, 0))
                        eT = etp.tile([128, iw], F32R, tag="et", bufs=3)
                        nc.scalar.activation(eT, eps_, AF.Exp,
                                             bias=ebt[:, jt:jt + 1], scale=1.0)
                        nc.tensor.matmul(pamt[ici], vT[:, jt, :], eT,
                                         start=(jt == 0), stop=(jt == NJT - 1))

            # pam normalize (r = gamma_pam / s), sa = pam_u*r + feat1
            for ici, (i0, iw) in enumerate(IC):
                r32 = sm.tile([1, iw], F32, tag="r32")
                nc.vector.reciprocal(r32, pamt[ici][64:65, :])
                rr = sm.tile([1, iw], F32R, tag="rr")
                nc.vector.tensor_scalar_mul(rr, r32, cst[0:1, 0:1])
                rbc = etp.tile([64, iw], F32R, tag="camt", bufs=2, name="rbc")
                nc.gpsimd.partition_broadcast(rbc, rr)
                tmpa = etp.tile([64, iw], F32R, tag="camt", bufs=2, name="tmpa")
                nc.vector.tensor_tensor(tmpa, pamt[ici][0:64, :], rbc, ALU.mult)
                r0, nr = i0 // W, iw // W
                nc.vector.tensor_tensor(
                    sabuf[0:64, r0:r0 + nr, 1:65],
                    tmpa[:, :].rearrange("p (r c) -> p r c", c=W),
                    feat[0:64, i0:i0 + iw].rearrange("p (r c) -> p r c", c=W),
                    ALU.add)
            nc.vector.tensor_scalar_mul(sabuf[0:64, 0, 1:65],
                                        sabuf[0:64, 0, 1:65], hmt[:, 0:1])
            nc.vector.tensor_scalar_mul(sabuf[0:64, 33, 1:65],
                                        sabuf[0:64, 33, 1:65], hmt[:, 1:2])
            nc.sync.dma_start(out=sabuf[64:128, 0:33, :],
                              in_=sabuf[0:64, 1:34, :])

            mva = conv2(sabuf, y2a, "sta")

            # ---- AR2 (combined stats of both branches)
            ar2s = sm.tile([64, 4], F32, tag="ar2s")
            t_b = sm.tile([64, 1], F32, tag="t_b")
            for col, mv in ((0, mva), (2, mvb)):
                nc.vector.tensor_scalar_mul(ar2s[:, col:col + 1], mv[:, 0:1],
                                            float(MY))
                nc.vector.tensor_tensor(t_b, mv[:, 0:1], mv[:, 0:1], ALU.mult)
                nc.vector.tensor_tensor(t_b, mv[:, 1:2], t_b, ALU.add)
                nc.vector.tensor_scalar_mul(ar2s[:, col + 1:col + 2], t_b,
                                            float(MY))
            ar2_in = dram.tile([64, 4], F32, tag="ar2_in")
            ar2_out = dram.tile([64, 4], F32, tag="ar2_out")
            nc.sync.dma_start(out=ar2_in[:, :], in_=ar2s)
            nc.gpsimd.collective_compute(
                "AllReduce", ALU.add, replica_groups=[list(range(NCORES))],
                ins=[ar2_in.opt()], outs=[ar2_out.opt()])
            gl2 = sm.tile([64, 4], F32, tag="gl2")
            nc.sync.dma_start(out=gl2, in_=ar2_out[:, :])
            sca, sha = bn_coeffs(gl2[:, 0:2], "bna")
            scb, shb = bn_coeffs(gl2[:, 2:4], "bnb")

            # ---- relu + sum + conv8
            ra = big.tile([64, MY], F32R, tag="ra")
            rb = big.tile([64, MY], F32R, tag="rb")
            nc.scalar.activation(ra, y2a, AF.Relu, bias=sha, scale=sca)
            nc.scalar.activation(rb, y2b, AF.Relu, bias=shb, scale=scb)
            nc.vector.tensor_tensor(fsum[0:64, :], ra, rb, ALU.add)
            for blk in range(2):
                for T in range(4):
                    ops_ = ps.tile([128, 512], F32, tag="ps")
                    nc.tensor.matmul(ops_, w8t[:, blk * 128:(blk + 1) * 128],
                                     fsum[:, T * 512:(T + 1) * 512],
                                     start=True, stop=True)
                    osb = obp.tile([128, 512], F32, tag="osb")
                    nc.vector.tensor_copy(osb, ops_)
                    nc.sync.dma_start(
                        out=out[blk * 128:(blk + 1) * 128,
                                T * 512:(T + 1) * 512],
                        in_=osb)
    nc.finalize()
    return nc


_NC_CACHE = {}


def kernel(**inputs):
    if "nc" not in _NC_CACHE:
        _NC_CACHE["nc"] = _build()
    nc = _NC_CACHE["nc"]
    x = np.asarray(inputs["x"], np.float32)
    in_maps = _prep_core_inputs(
        x, np.asarray(inputs["w1"]), np.asarray(inputs["bn_g"]),
        np.asarray(inputs["bn_b"]), np.asarray(inputs["wq"]),
        np.asarray(inputs["bq"]), np.asarray(inputs["wk"]),
        np.asarray(inputs["bk"]), np.asarray(inputs["wv"]),
        np.asarray(inputs["bv"]), np.asarray(inputs["gamma_pam"]),
        np.asarray(inputs["gamma_cam"]), np.asarray(inputs["w2"]),
        np.asarray(inputs["w8"]), np.asarray(inputs["b8"]))
    res = run_bass_kernel_spmd(nc, in_maps, list(range(NCORES)))
    out = np.zeros((B, CO, H, W), np.float32)
    for c in range(NCORES):
        b, h = divmod(c, 2)
        out[b, :, 32 * h:32 * h + 32, :] = \
            res.results[c]["out"].reshape(CO, 32, W)
    return out
